# revision 7
# baseline (speedup 1.0000x reference)
"""BSBR attention kernel for 8 Trainium2 NeuronCores.

Sharding: data-parallel over batch (B=2) x tensor-parallel over heads
(16 heads -> 4 heads per core). Core c handles batch c//4, head group c%4.
Each core computes its 4 heads' attention output and the partial output
projection (attn_heads @ Wo[:, head_dims].T); the host sums the 4 partials
per batch and adds bo.

Device-side layout choices:
- x is fed pre-transposed (xT [D, S]) so projection contractions read it
  directly; weights are fed pre-transposed/sliced per core.
- QT/KT produced in [dh, s] layout, V in [s, dh] (+ ones column for the
  softmax denominator trick). All PE transposes are avoided in the hot
  loop (DMA xbar transposes instead) so the PE HAM clock stays at 2.4GHz.
- Local attention uses the S^T = K Q^T formulation: softmax runs along
  the free axis after a mask-add, exp on ACT, and U = P^T V plus the
  row-sum via a ones column appended to V.
- Inter-chunk retrieval keeps F and retrieved in row layouts for the
  probability mix and converts layouts via small DRAM-bounce DMAs.
"""

import numpy as np

try:
    import concourse.bass as bass
except ImportError:
    import sys

    sys.path.insert(0, "/opt/trn_rl_repo")
    import concourse.bass as bass

import ml_dtypes
from contextlib import ExitStack

import concourse.tile as tile
from concourse import mybir
from concourse.bass_utils import run_bass_kernel_spmd

BF16 = ml_dtypes.bfloat16
B, S, D, H, CS = 2, 4096, 1024, 16, 128
HD = D // H          # 64
C = S // CS          # 32
NCORES = 8
DHC = 4 * HD         # 256 head dims per core
KB = D // 128        # 8 contraction blocks
NEG = -1e9

bf = mybir.dt.bfloat16
f32 = mybir.dt.float32
Exp = mybir.ActivationFunctionType.Exp


def _split_heavy_waits(nc, keep=1):
    """The walrus build in this container rejects >keep sync waits on several
    instruction encodings (Drain/TPB_CTRL, DmaTransposeAnt, ...). Hoist excess
    waits onto preceding NoOps on the same engine — the sequencer executes
    them in order, so semantics are preserved."""
    for fn in nc.m.functions:
        for bb in fn.blocks:
            insts = bb.instructions
            i = 0
            while i < len(insts):
                inst = insts[i]
                si = inst.sync_info
                if si is not None and si.on_wait and len(si.on_wait) > keep:
                    waits = list(si.on_wait)
                    head, tail = waits[:-keep], waits[-keep:]
                    for j, w in enumerate(head):
                        nop = mybir.InstNoOp(
                            name=nc.get_next_instruction_name(), ins=[], outs=[]
                        )
                        nop.engine = inst.engine
                        nop.sync_info = mybir.SyncInfo(on_wait=[w], on_update=[])
                        nc.register_instruction(nop, overwrite=True)
                        insts.insert(i + j, nop)
                    inst.sync_info = mybir.SyncInfo(
                        on_wait=tail, on_update=list(si.on_update)
                    )
                    i += len(head)
                i += 1


def _build_program():
    nc = bass.Bass("TRN2", debug=False, num_devices=NCORES)

    ap = {}
    def din(name, shape, dtype):
        ap[name] = nc.dram_tensor(name, shape, dtype, kind="ExternalInput").ap()

    din("xT", [D, S], bf)
    for w in ("wqT", "wkT", "wvT", "wrT", "whT"):
        din(w, [D, DHC], bf)
    din("woT", [DHC, D], bf)
    din("bias", [128, 8], f32)
    din("bvrow", [1, DHC + 4], bf)
    din("maskT", [128, 128], f32)
    din("cmask4", [128, C], f32)
    out_ap = nc.dram_tensor("out", [S, D], f32, kind="ExternalOutput").ap()

    with tile.TileContext(nc) as tc, ExitStack() as ctx:
        const = ctx.enter_context(tc.tile_pool(name="const", bufs=1))
        wpool = ctx.enter_context(tc.tile_pool(name="wpool", bufs=1))
        big = ctx.enter_context(tc.tile_pool(name="big", bufs=1))
        dram = ctx.enter_context(tc.tile_pool(name="dramp", bufs=1, space="DRAM"))

        # ---- constants ----
        maskT_sb = const.tile([128, 128], f32)
        nc.sync.dma_start(maskT_sb[:], ap["maskT"][:])
        cmask_sb = const.tile([128, C], f32)
        nc.sync.dma_start(cmask_sb[:], ap["cmask4"][:])
        bias_sb = const.tile([128, 8], f32)
        nc.sync.dma_start(bias_sb[:], ap["bias"][:])
        bvrow_sb = const.tile([1, DHC + 4], bf)
        nc.sync.dma_start(bvrow_sb[:], ap["bvrow"][:])
        onesrow_sb = const.tile([1, 128], bf)
        nc.vector.memset(onesrow_sb[:], 1.0)
        onescol_sb = const.tile([128, 1], bf)
        nc.vector.memset(onescol_sb[:], 1.0)

        # ---- x (already transposed on host) ----
        xt_sb = [big.tile([128, S], bf, name=f"xt{k}") for k in range(KB)]
        for k in range(KB):
            eng = nc.sync if k % 2 == 0 else nc.scalar
            eng.dma_start(xt_sb[k][:], ap["xT"][k * 128 : (k + 1) * 128, :])

        # ---- weights (SBUF layout [128, kblock, outdim]) ----
        def load_w(name, eng):
            t = wpool.tile([128, KB, DHC], bf, name=f"{name}_sb")
            eng.dma_start(t[:], ap[name].rearrange("(k p) d -> p k d", p=128))
            return t

        wq_sb = load_w("wqT", nc.sync)
        wk_sb = load_w("wkT", nc.scalar)
        wv_sb = load_w("wvT", nc.sync)
        wr_sb = load_w("wrT", nc.scalar)
        wh_sb = load_w("whT", nc.sync)
        wo_sb = wpool.tile([128, 2, D], bf)
        nc.scalar.dma_start(wo_sb[:], ap["woT"].rearrange("(k p) j -> p k j", p=128))

        # ---- persistent activations ----
        qt_sb = [big.tile([128, S], bf, name=f"qt{m}") for m in range(2)]
        kt_sb = [big.tile([128, S], bf, name=f"kt{m}") for m in range(2)]
        v_sb = [big.tile([128, 4 * 65], bf, name=f"v{i}") for i in range(C)]
        rt_sb = big.tile([128, 2, C], bf)
        ht_sb = big.tile([128, 2, C], bf)
        expct_sb = big.tile([128, C], bf)
        crecip_sb = big.tile([128, 1], f32)
        fnat_sb = [big.tile([128, C * 64], bf, name=f"fnat{p}") for p in range(2)]
        frows_sb = big.tile([128, 64 * 64], bf)
        retrrows_sb = big.tile([128, 64 * 64], bf)
        retrt_sb = [big.tile([128, C * 64], bf, name=f"retrt{p}") for p in range(2)]
        attnt_sb = [big.tile([128, S], bf, name=f"attnt{p}") for p in range(2)]

        fb = dram.tile([2, 2, 64, C, 64], bf)   # (pair, h2, d, c', e)
        rbt = dram.tile([4, C, 64, 64], bf)     # (head, c, d, e)

        # ---- projections ----
        with tc.tile_pool(name="pjp", bufs=2, space="PSUM") as pjp:
            # QT / KT: [dh, s] layout
            for w_sb, dst, bcol in ((wq_sb, qt_sb, 0), (wk_sb, kt_sb, 2)):
                for m in range(2):
                    for n in range(8):
                        ps = pjp.tile([128, 512], f32, tag="pj")
                        for k in range(KB):
                            nc.tensor.matmul(
                                ps[:],
                                w_sb[:, k, m * 128 : (m + 1) * 128],
                                xt_sb[k][:, n * 512 : (n + 1) * 512],
                                start=(k == 0),
                                stop=(k == KB - 1),
                            )
                        nc.scalar.add(
                            dst[m][:, n * 512 : (n + 1) * 512],
                            ps[:],
                            bias_sb[:, bcol + m : bcol + m + 1],
                        )

            # V natural [s, dh] (+ ones col per head via rank-1 trick)
            for i in range(C):
                ps = pjp.tile([128, DHC + 4], f32, tag="pv")
                nc.tensor.matmul(
                    ps[:],
                    onesrow_sb[:],
                    bvrow_sb[:],
                    start=True,
                    stop=False,
                    skip_group_check=True,
                )
                for k in range(KB):
                    nc.tensor.matmul(
                        ps[:, 0:DHC],
                        xt_sb[k][:, i * 128 : (i + 1) * 128],
                        wv_sb[:, k, :],
                        start=False,
                        stop=(k == KB - 1),
                        skip_group_check=True,
                    )
                vr = v_sb[i].rearrange("p (h e) -> p h e", e=65)
                nc.vector.tensor_copy(
                    vr[:, :, 0:64], ps[:, 0:DHC].rearrange("p (h e) -> p h e", e=64)
                )
                nc.vector.tensor_copy(
                    vr[:, :, 64:65],
                    ps[:, DHC : DHC + 4].rearrange("p (h e) -> p h e", e=1),
                )

            # r/h meta projections: [dh, c] layout
            crepr = [
                xt_sb[k].rearrange("p (c cs) -> p c cs", cs=CS)[:, :, CS - 1]
                for k in range(KB)
            ]
            for w_sb, dst, bcol in ((wr_sb, rt_sb, 4), (wh_sb, ht_sb, 6)):
                for m in range(2):
                    ps = pjp.tile([128, C], f32, tag="pr")
                    for k in range(KB):
                        nc.tensor.matmul(
                            ps[:],
                            w_sb[:, k, m * 128 : (m + 1) * 128],
                            crepr[k],
                            start=(k == 0),
                            stop=(k == KB - 1),
                        )
                    nc.scalar.add(
                        dst[:, m, :], ps[:], bias_sb[:, bcol + m : bcol + m + 1]
                    )

        # ---- K natural (DMA xbar transpose) + F = k^T v per chunk ----
        with (
            tc.tile_pool(name="kns", bufs=4) as kns,
            tc.tile_pool(name="fps", bufs=2, space="PSUM") as fpsp,
        ):
            for i in range(C):
                for p in range(2):
                    knat = kns.tile([128, 128], bf, tag="knat")
                    nc.sync.dma_start(
                        knat[:], kt_sb[p][:, i * 128 : (i + 1) * 128], transpose=True
                    )
                    fps = fpsp.tile([128, 64], f32, tag="fps")
                    vr = v_sb[i].rearrange("p (h e) -> p h e", e=65)
                    for h2 in range(2):
                        nc.tensor.matmul(
                            fps[64 * h2 : 64 * h2 + 64, :],
                            knat[:, 64 * h2 : 64 * h2 + 64],
                            vr[:, 2 * p + h2, 0:64],
                            start=True,
                            stop=True,
                            skip_group_check=True,
                        )
                    nc.vector.tensor_copy(fnat_sb[p][:, i * 64 : (i + 1) * 64], fps[:])

        # F -> F_rows via DRAM bounce
        for p in range(2):
            for h2 in range(2):
                nc.sync.dma_start(
                    fb[p, h2],
                    fnat_sb[p][64 * h2 : 64 * h2 + 64, :].rearrange(
                        "d (c e) -> d c e", e=64
                    ),
                )
        for h in range(4):
            p, h2 = divmod(h, 2)
            nc.scalar.dma_start(
                frows_sb[32 * h : 32 * h + 32, :].rearrange("c (d e) -> c d e", e=64),
                fb[p, h2].rearrange("d c e -> c d e"),
            )

        # chunk scores -> probs -> retrieved rows
        with tc.tile_pool(name="csp", bufs=1, space="PSUM") as cspp:
            csp = cspp.tile([128, C], f32, tag="csp")
            for h in range(4):
                hb = 64 * (h % 2)
                nc.tensor.matmul(
                    csp[32 * h : 32 * h + 32, :],
                    ht_sb[hb : hb + 64, h // 2, :],
                    rt_sb[hb : hb + 64, h // 2, :],
                    start=True,
                    stop=True,
                    skip_group_check=True,
                    tile_position=(hb, 32 * h),
                )
            nc.vector.tensor_add(csp[:], csp[:], cmask_sb[:])
            nc.scalar.activation(expct_sb[:], csp[:], Exp, scale=0.125)
            csums = cspp.tile([128, 1], f32, tag="csums")
            for h in range(4):
                nc.tensor.matmul(
                    csums[32 * h : 32 * h + 32, :],
                    expct_sb[32 * h : 32 * h + 32, :],
                    onescol_sb[32 * h : 32 * h + 32, :],
                    start=True,
                    stop=True,
                    skip_group_check=True,
                    tile_position=(32 * h, 32 * h),
                )
            nc.vector.reciprocal(crecip_sb[:], csums[:])

            with tc.tile_pool(name="mixp", bufs=2, space="PSUM") as mixp:
                for nb in range(8):
                    mps = mixp.tile([128, 512], f32, tag="mps")
                    for h in range(4):
                        nc.tensor.matmul(
                            mps[32 * h : 32 * h + 32, :],
                            expct_sb[32 * h : 32 * h + 32, :],
                            frows_sb[32 * h : 32 * h + 32, nb * 512 : (nb + 1) * 512],
                            start=True,
                            stop=True,
                            skip_group_check=True,
                            tile_position=(32 * h, 32 * h),
                        )
                    nc.scalar.mul(
                        retrrows_sb[:, nb * 512 : (nb + 1) * 512],
                        mps[:],
                        crecip_sb[:, 0:1],
                    )

        # retrieved rows -> retrT via DRAM bounce
        for h in range(4):
            nc.sync.dma_start(
                rbt[h],
                retrrows_sb[32 * h : 32 * h + 32, :].rearrange("c (d e) -> c d e", e=64),
            )
        for p in range(2):
            for h2 in range(2):
                nc.scalar.dma_start(
                    retrt_sb[p][64 * h2 : 64 * h2 + 64, :].rearrange(
                        "d (c e) -> d c e", e=64
                    ),
                    rbt[2 * p + h2].rearrange("c d e -> d c e"),
                )

        # ---- local attention + long-term per chunk-head ----
        with (
            tc.tile_pool(name="stp", bufs=3, space="PSUM") as stp,
            tc.tile_pool(name="ulp", bufs=2, space="PSUM") as ulp,
            tc.tile_pool(name="ltp", bufs=2, space="PSUM") as ltp,
            tc.tile_pool(name="exps", bufs=4) as expp,
            tc.tile_pool(name="smalls", bufs=4) as smalls,
            tc.tile_pool(name="anp", bufs=2) as anp,
        ):
            for i in range(C):
                sl = slice(i * 128, (i + 1) * 128)
                attn_nat = anp.tile([128, DHC], bf, tag="attn_nat")
                sts, exps, us, lts = [], [], [], []
                # all S^T matmuls first: PE never waits on the softmax path
                for h in range(4):
                    hp, hb = h // 2, 64 * (h % 2)
                    st = stp.tile([128, 128], f32, tag="st")
                    nc.tensor.matmul(
                        st[:], kt_sb[hp][hb : hb + 64, sl], qt_sb[hp][hb : hb + 64, sl],
                        start=True, stop=True,
                    )
                    sts.append(st)
                for h in range(4):
                    hp, hb = h // 2, 64 * (h % 2)
                    lt = ltp.tile([128, 64], f32, tag="lt")
                    nc.tensor.matmul(
                        lt[:],
                        qt_sb[hp][hb : hb + 64, sl],
                        retrt_sb[hp][hb : hb + 64, i * 64 : (i + 1) * 64],
                        start=True, stop=True,
                    )
                    lts.append(lt)
                for h in range(4):
                    nc.vector.tensor_add(sts[h][:], sts[h][:], maskT_sb[:])
                    expst = expp.tile([128, 128], bf, tag="expst")
                    nc.scalar.activation(expst[:], sts[h][:], Exp, scale=0.125)
                    exps.append(expst)
                for h in range(4):
                    u = ulp.tile([128, 65], f32, tag="u")
                    nc.tensor.matmul(
                        u[:], exps[h][:], v_sb[i][:, h * 65 : (h + 1) * 65],
                        start=True, stop=True,
                    )
                    us.append(u)
                for h in range(4):
                    rcp = smalls.tile([128, 1], f32, tag="rcp")
                    nc.vector.reciprocal(rcp[:], us[h][:, 64:65])
                    un = smalls.tile([128, 64], bf, tag="un")
                    nc.scalar.mul(un[:], us[h][:, 0:64], rcp[:, 0:1])
                    nc.vector.tensor_add(
                        attn_nat[:, h * 64 : (h + 1) * 64], un[:], lts[h][:]
                    )
                for half in range(2):
                    nc.scalar.dma_start(
                        attnt_sb[half][:, sl],
                        attn_nat[:, half * 128 : (half + 1) * 128],
                        transpose=True,
                    )

        # ---- output projection ----
        with (
            tc.tile_pool(name="outp", bufs=2, space="PSUM") as outp,
            tc.tile_pool(name="outs", bufs=3) as outs,
        ):
            for i in range(C):
                osb = outs.tile([128, D], f32, tag="osb")
                for nb in range(2):
                    ops = outp.tile([128, 512], f32, tag="ops")
                    for p in range(2):
                        nc.tensor.matmul(
                            ops[:],
                            attnt_sb[p][:, i * 128 : (i + 1) * 128],
                            wo_sb[:, p, nb * 512 : (nb + 1) * 512],
                            start=(p == 0),
                            stop=(p == 1),
                        )
                    if nb == 0:
                        nc.vector.tensor_copy(osb[:, nb * 512 : (nb + 1) * 512], ops[:])
                    else:
                        nc.scalar.copy(osb[:, nb * 512 : (nb + 1) * 512], ops[:])
                eng = nc.sync if i % 2 == 0 else nc.scalar
                eng.dma_start(out_ap[i * 128 : (i + 1) * 128, :], osb[:])

    _split_heavy_waits(nc)
    return nc


_CACHE = {}


def _get_program():
    if "nc" not in _CACHE:
        _CACHE["nc"] = _build_program()
    return _CACHE["nc"]


def _make_in_maps(inputs):
    hs = np.asarray(inputs["hidden_states"], dtype=np.float32)
    W = {k: np.asarray(inputs[k], dtype=np.float32) for k in
         ("Wq", "Wk", "Wv", "Wo", "Wr", "Wh")}
    bvec = {k: np.asarray(inputs[k], dtype=np.float32) for k in
            ("bq", "bk", "bv", "bo", "br", "bh")}

    # local mask in [t, s] layout: keep t >= s
    tt, ss = np.meshgrid(np.arange(128), np.arange(128), indexing="ij")
    maskT = np.where(tt >= ss, 0.0, NEG).astype(np.float32)
    # chunk mask in [c', c] layout: keep c' >= c; tiled x4 heads
    cc2, cc = np.meshgrid(np.arange(C), np.arange(C), indexing="ij")
    cmask = np.where(cc2 >= cc, 0.0, NEG).astype(np.float32)
    cmask4 = np.tile(cmask, (4, 1)).astype(np.float32)

    xT_bf = [np.ascontiguousarray(hs[b].T).astype(BF16) for b in range(B)]

    in_maps = []
    for c in range(NCORES):
        b, hg = divmod(c, 4)
        sl = slice(hg * DHC, (hg + 1) * DHC)
        bias = np.stack(
            [
                bvec["bq"][sl][:128], bvec["bq"][sl][128:],
                bvec["bk"][sl][:128], bvec["bk"][sl][128:],
                bvec["br"][sl][:128], bvec["br"][sl][128:],
                bvec["bh"][sl][:128], bvec["bh"][sl][128:],
            ],
            axis=1,
        ).astype(np.float32)
        bvrow = np.concatenate([bvec["bv"][sl], np.ones(4, np.float32)])[None, :].astype(BF16)
        in_maps.append(
            {
                "xT": xT_bf[b],
                "wqT": np.ascontiguousarray(W["Wq"][sl, :].T).astype(BF16),
                "wkT": np.ascontiguousarray(W["Wk"][sl, :].T).astype(BF16),
                "wvT": np.ascontiguousarray(W["Wv"][sl, :].T).astype(BF16),
                "wrT": np.ascontiguousarray(W["Wr"][sl, :].T).astype(BF16),
                "whT": np.ascontiguousarray(W["Wh"][sl, :].T).astype(BF16),
                "woT": np.ascontiguousarray(W["Wo"][:, sl].T).astype(BF16),
                "bias": bias,
                "bvrow": bvrow,
                "maskT": maskT,
                "cmask4": cmask4,
            }
        )
    return in_maps, bvec["bo"]


def kernel(**inputs):
    nc = _get_program()
    in_maps, bo = _make_in_maps(inputs)
    res = run_bass_kernel_spmd(nc, in_maps, core_ids=list(range(NCORES)))
    _CACHE["last_results"] = res
    out = np.zeros((B, S, D), np.float32)
    for c in range(NCORES):
        out[c // 4] += res.results[c]["out"]
    out += bo[None, None, :]
    return out
